# revision 3
# baseline (speedup 1.0000x reference)
"""Trainium2 Bass kernel for Conv2D(sum of 20 1x1 convs) + QwenRMSNorm.

Math: y = einsum("bsi,loi->bso", x, conv_w) / L ; out = rmsnorm(y) * norm_w.
Since x does not depend on l, the 20-matrix contraction collapses to a single
matmul with W = sum_l conv_w[l] / L. Host pre-sums/transposes/casts the weight
(one [H,H] bf16 matrix) and lays out x as token-sharded, hidden-major bf16
slabs; the 8 NeuronCores each run matmul (bf16, fp32 accum) + RMSNorm on their
2048 tokens. All device compute is token-local; no collectives.

Schedule (from NTFF profile analysis on hardware):
- All input DMAs ride ONE HWDGE ring in exact consumption order (cross-ring
  round-robin starves small transfers); outputs ride the other ring.
- The PE clock (HAM) is pre-warmed with dummy matmuls during the DMA lead-in,
  and the first tblocks walk the (tblock, ib) grid as an anti-diagonal
  wavefront so PE demand matches the w-chunk / x-slab arrival order.
- ib-outer matmul order shares each LDWEIGHTS across the two PSUM halves
  (measured 259ns/MM with per-MM loads vs 216ns shared).
- RMSNorm: one full-row Square+accumulate -> Sqrt -> reciprocal -> two
  scalar_tensor_tensor ops; last tblock's store is split to pull in the
  final DMA completion.
"""

import numpy as np
import ml_dtypes
from contextlib import ExitStack

import concourse.bass as bass
import concourse.mybir as mybir
import concourse.tile as tile
from concourse.bass_utils import run_bass_kernel_spmd

N_CORES = 8
B, S, H, L = 4, 4096, 1024, 20
TOK = B * S               # 16384 tokens
TPC = TOK // N_CORES      # 2048 tokens per core
TB = TPC // 128           # 16 token-blocks of 128 per core
KB = H // 128             # 8 contraction blocks
NOH = H // 512            # 2 psum halves of the output row
EPS = 1e-6

BF16 = mybir.dt.bfloat16
F32 = mybir.dt.float32
AF = mybir.ActivationFunctionType
OP = mybir.AluOpType

PHA = 3      # wavefront tblocks
N_WARM = 6   # clock pre-warm matmuls (cover preamble->first-data window)

_BUILT = None        # cached Bass program
LAST_RESULTS = None  # BassKernelResults of the most recent run (for harness)


def _legalize_multiwait(nc):
    """The walrus build here encodes exactly one semaphore wait per 64B
    instruction (NEURON_ISA_TPB_EVENTS has a single wait slot) and errors on
    Tile's multi-wait instructions.  Split surplus waits into standalone
    EVENT_SEMAPHORE instructions on the same engine, placed directly before
    the original instruction (same sequencer stream -> same semantics)."""
    n_ev = 0
    for f in nc.m.functions:
        for blk in f.blocks:
            insts = blk.instructions
            out = []
            changed = False
            for inst in list(insts):
                si = getattr(inst, "sync_info", None)
                waits = list(si.on_wait) if si is not None else []
                if len(waits) > 1:
                    changed = True
                    updates = list(si.on_update)
                    for w in waits[:-1]:
                        ev = mybir.InstEventSemaphore(
                            name=f"{inst.name}-sw{n_ev}", ins=[], outs=[])
                        n_ev += 1
                        ev.engine = inst.engine
                        ev.sync_info = mybir.SyncInfo(on_wait=[w], on_update=[])
                        out.append(ev)
                    inst.sync_info = mybir.SyncInfo(
                        on_wait=[waits[-1]], on_update=updates)
                out.append(inst)
            if changed:
                insts.clear()
                insts.extend(out)


def _build(loop_k=1):
    nc = bass.Bass()
    xt_h = nc.dram_tensor("xt", [TB, 128, KB, 128], BF16, kind="ExternalInput")
    wt_h = nc.dram_tensor("wt", [128, KB, H], BF16, kind="ExternalInput")
    nw_h = nc.dram_tensor("nw", [H], F32, kind="ExternalInput")
    out_h = nc.dram_tensor("out", [TPC, H], F32, kind="ExternalOutput")

    with tile.TileContext(nc) as tc, ExitStack() as ctx:
        xpool = ctx.enter_context(tc.tile_pool(name="x", bufs=TB))
        wpool = ctx.enter_context(tc.tile_pool(name="w", bufs=1))
        cpool = ctx.enter_context(tc.tile_pool(name="consts", bufs=1))
        opool = ctx.enter_context(tc.tile_pool(name="out", bufs=6))
        spool = ctx.enter_context(tc.tile_pool(name="scratch", bufs=4))
        stats = ctx.enter_context(tc.tile_pool(name="stats", bufs=12))
        psum = ctx.enter_context(tc.tile_pool(name="psum", bufs=4, space="PSUM"))
        wrm = ctx.enter_context(tc.tile_pool(name="warm", bufs=1))

        w_sb = wpool.tile([128, KB, H], BF16)
        x_sb = [xpool.tile([128, KB, 128], BF16, name=f"xs{tt}", tag="xsb")
                for tt in range(TB)]

        # ALL inputs on one ring (ACT), in exact consumption order; outs get
        # the SP ring, which stays empty until the first store.
        def win(ib):
            return nc.scalar.dma_start(out=w_sb[:, ib:ib + 1, :],
                                       in_=wt_h[:, ib:ib + 1, :])

        nc.scalar.dma_start(out=x_sb[0][:, 0:2, :], in_=xt_h[0, :, 0:2, :])
        win(0)
        nc.scalar.dma_start(out=x_sb[0][:, 2:KB, :], in_=xt_h[0, :, 2:KB, :])
        nc.scalar.dma_start(out=x_sb[1], in_=xt_h[1])
        win(1)
        nc.scalar.dma_start(out=x_sb[2], in_=xt_h[2])
        w_last = None
        for ib in range(2, KB):
            w_last = win(ib)
        # x3..x15 ride the otherwise-idle SWDGE ring so their 13 descriptor
        # generations (~0.65us each) stop saturating the ACT sequencer, which
        # must run the norm chains in the same window (late norms -> late
        # PSUM releases -> the periodic tblock-start stalls). The explicit
        # dep on the last w chunk keeps them from stealing HBM bandwidth
        # during the lead-in ramp.
        for tt in range(PHA, TB):
            xd = nc.gpsimd.dma_start(out=x_sb[tt], in_=xt_h[tt])
            tile.add_dep_helper(
                xd.ins, w_last.ins, sync=True,
                reason="defer late x slabs behind the w stream")

        nw_sb = cpool.tile([128, H], F32)
        zero_sb = cpool.tile([128, 1], F32)
        nc.vector.memset(zero_sb, 0.0)
        eps_sb = cpool.tile([128, 1], F32)
        nc.vector.memset(eps_sb, EPS)

        dummy = wrm.tile([128, 512], BF16)
        nc.vector.memset(dummy, 0.0)
        wp = psum.tile([128, 512], F32, name="warmps", tag="yp")

        def fill(n):
            for _ in range(n):
                nc.tensor.matmul(wp, dummy[:, 0:128], dummy,
                                 start=True, stop=True, skip_group_check=True)

        fill(N_WARM)

        nc.gpsimd.dma_start(
            out=nw_sb, in_=bass.AP(tensor=nw_h, offset=0, ap=[[0, 128], [1, H]]))

        def mm(yp, tt, ib, skip=False):
            for oh in range(NOH):
                nc.tensor.matmul(
                    yp[:, oh * 512:(oh + 1) * 512],
                    x_sb[tt][:, ib, :],
                    w_sb[:, ib, oh * 512:(oh + 1) * 512],
                    start=(ib == 0), stop=(ib == KB - 1),
                    skip_group_check=skip)

        def norm_and_out(tt, yp, split_out=1):
            sq = spool.tile([128, H], BF16)
            ssum = stats.tile([128, 1], F32)
            nc.scalar.activation(out=sq, in_=yp, func=AF.Square,
                                 bias=zero_sb, accum_out=ssum)
            std = stats.tile([128, 1], F32)
            nc.scalar.activation(out=std, in_=ssum, func=AF.Sqrt,
                                 bias=eps_sb, scale=1.0 / H)
            rstd = stats.tile([128, 1], F32)
            nc.vector.reciprocal(out=rstd, in_=std)
            o_sb = opool.tile([128, H], F32)
            for oh in range(NOH):
                sl = slice(oh * 512, (oh + 1) * 512)
                nc.vector.scalar_tensor_tensor(
                    out=o_sb[:, sl], in0=yp[:, sl], scalar=rstd,
                    in1=nw_sb[:, sl], op0=OP.mult, op1=OP.mult)
            row = out_h[tt * 128:(tt + 1) * 128, :]
            step = H // split_out
            for c in range(split_out):
                sl = slice(c * step, (c + 1) * step)
                nc.sync.dma_start(out=row[:, sl], in_=o_sb[:, sl])

        first = True
        for k in range(loop_k):
            if first:
                ypA = [psum.tile([128, H], F32, name=f"ypA{t}", tag="yp")
                       for t in range(PHA)]
                # anti-diagonal wavefront: pair (tt, ib) emitted at step
                # s = tt + ib, matching the slab/chunk arrival order
                for s in range(PHA + KB - 1):
                    for tt in reversed(range(PHA)):
                        ib = s - tt
                        if 0 <= ib < KB:
                            mm(ypA[tt], tt, ib, skip=True)
                    if s < 4:
                        fill(1)  # absorb arrival jitter, keep HAM warm
                for tt in range(PHA):
                    norm_and_out(tt, ypA[tt])
                rest = range(PHA, TB)
                first = False
            else:
                rest = range(TB)
            for tt in rest:
                yp = psum.tile([128, H], F32, name="ypB", tag="yp")
                for ib in range(KB):
                    mm(yp, tt, ib)
                last = (k == loop_k - 1) and (tt == TB - 1)
                norm_and_out(tt, yp, split_out=(2 if last else 1))

    _legalize_multiwait(nc)
    return nc


def host_prep(x, conv_w, norm_w):
    """Shard + lay out the full inputs into per-core device input maps."""
    bf16 = ml_dtypes.bfloat16

    # Collapse the 20 1x1 convs: W[o,i] = sum_l conv_w[l,o,i] / L
    w = np.asarray(conv_w).sum(axis=0) * (1.0 / L)          # [H(o), H(i)] f32
    # wt[p, ib, o] = W[o, ib*128+p]
    wt = np.ascontiguousarray(
        w.reshape(H, KB, 128).transpose(2, 1, 0).astype(bf16))
    nw = np.ascontiguousarray(np.asarray(norm_w), dtype=np.float32)

    x2d = np.asarray(x).reshape(TOK, H)
    xbf = x2d.astype(bf16)

    in_maps = []
    for c in range(N_CORES):
        xc = xbf[c * TPC:(c + 1) * TPC]                      # [TPC, H]
        # xt[tt, p, ib, t] = xc[tt*128+t, ib*128+p]
        xtc = np.ascontiguousarray(
            xc.reshape(TB, 128, KB, 128).transpose(0, 3, 2, 1))
        in_maps.append({"xt": xtc, "wt": wt, "nw": nw})
    return in_maps


def kernel(x, conv_w, norm_w):
    global _BUILT, LAST_RESULTS
    if _BUILT is None:
        _BUILT = _build()
    nc = _BUILT

    x = np.asarray(x)
    out_dtype = x.dtype
    in_maps = host_prep(x, conv_w, norm_w)

    res = run_bass_kernel_spmd(nc, in_maps, core_ids=list(range(N_CORES)))
    LAST_RESULTS = res

    out = np.concatenate([r["out"] for r in res.results], axis=0)
    return out.reshape(B, S, H).astype(out_dtype, copy=False)
